# revision 6
# baseline (speedup 1.0000x reference)
import sys
if '/opt/trn_rl_repo' not in sys.path:
    sys.path.insert(0, '/opt/trn_rl_repo')
import numpy as np
import ml_dtypes

import concourse.bass as bass
import concourse.bacc as bacc
import concourse.mybir as mybir
import concourse.tile as tile
from concourse import library_config
from concourse.masks import make_identity
from concourse.bass_utils import run_bass_kernel_spmd
from concourse._compat import cdiv

NCORE = 8
N_NODES = 100000
N_EDGES = 3200000
NODE_DIM = 128
HID = 16
NGRAPH = 256
NCLS = 3
NLOC = 12544            # nodes per core (98 tiles of 128); 8*12544 = 100352
NT = NLOC // 128        # 98 tiles per core
NQUAD = (NCORE * NLOC) // 4
MODS = [("mri", 256), ("cog", 64), ("clin", 32), ("gen", 512)]

F32 = mybir.dt.float32
BF16 = mybir.dt.bfloat16
I16 = mybir.dt.int16
AX = mybir.AxisListType
OP = mybir.AluOpType
ACT = mybir.ActivationFunctionType


def _build(Ks, totw, tot4):
    """Build the per-core SPMD program. Ks: per-tile slot counts (same for
    every core by construction), totw/tot4: flat widths of the index arrays."""
    nc = bacc.Bacc(num_swdge_queues=4)
    P = {}
    P['x16'] = nc.declare_dram_parameter("x16", [NLOC, NODE_DIM], BF16, isOutput=False)
    P['W16'] = nc.declare_dram_parameter("W16", [NODE_DIM, HID], BF16, isOutput=False)
    P['degT'] = nc.declare_dram_parameter("degT", [128, NT], F32, isOutput=False)
    P['gidxT'] = nc.declare_dram_parameter("gidxT", [128, NT], F32, isOutput=False)
    P['giota'] = nc.declare_dram_parameter("giota", [128, NGRAPH], F32, isOutput=False)
    P['brep'] = nc.declare_dram_parameter("brep", [128, HID], F32, isOutput=False)
    P['slotq'] = nc.declare_dram_parameter("slotq", [128, totw], I16, isOutput=False)
    P['subx4'] = nc.declare_dram_parameter("subx4", [128, tot4], F32, isOutput=False)
    P['iota4'] = nc.declare_dram_parameter("iota4", [128, 4 * max(Ks)], F32, isOutput=False)
    for m, fdim in MODS:
        P[m + 'T'] = nc.declare_dram_parameter(m + 'T', [fdim, NGRAPH], F32, isOutput=False)
        P[m + 'W'] = nc.declare_dram_parameter(m + 'W', [fdim, 4], F32, isOutput=False)
        P[m + 'b'] = nc.declare_dram_parameter(m + 'b', [4, 1], F32, isOutput=False)
    P['cW1'] = nc.declare_dram_parameter("cW1", [32, HID], F32, isOutput=False)
    P['cb1'] = nc.declare_dram_parameter("cb1", [HID, 1], F32, isOutput=False)
    P['cW2'] = nc.declare_dram_parameter("cW2", [HID, NCLS], F32, isOutput=False)
    P['cb2'] = nc.declare_dram_parameter("cb2", [NCLS, 1], F32, isOutput=False)
    out = nc.declare_dram_parameter("out", [NGRAPH, NCLS], F32, isOutput=True)

    tloc = nc.dram_tensor("tloc", [NLOC, HID], F32)
    table = nc.dram_tensor("table", [NCORE * NLOC, HID], F32, addr_space="Shared")
    poolloc = nc.dram_tensor("poolloc", [HID + 1, NGRAPH], F32)
    poolred = nc.dram_tensor("poolred", [HID + 1, NGRAPH], F32, addr_space="Shared")
    groups = [list(range(NCORE))]

    with tile.TileContext(nc) as tc:
        with tc.tile_pool(name="pers", bufs=1) as pp, \
             tc.tile_pool(name="sb", bufs=2) as sb, \
             tc.tile_pool(name="gat", bufs=4) as gb, \
             tc.tile_pool(name="ps", bufs=2, space="PSUM") as ps, \
             tc.tile_pool(name="pool_ps", bufs=1, space="PSUM") as pps:
            nc.gpsimd.load_library(library_config.mlp)

            # ---------- phase 1: local xw2 shard + AllGather table ----------
            xT = pp.tile([128, NLOC], BF16)
            nc.sync.dma_start_transpose(xT[:], P['x16'][:])
            Wt = pp.tile([NODE_DIM, HID], BF16)
            nc.sync.dma_start(out=Wt[:], in_=P['W16'][:])
            degT = pp.tile([128, NT], F32)
            nc.sync.dma_start(out=degT[:], in_=P['degT'][:])
            gidxT = pp.tile([128, NT], F32)
            nc.sync.dma_start(out=gidxT[:], in_=P['gidxT'][:])
            giota = pp.tile([128, NGRAPH], F32)
            nc.sync.dma_start(out=giota[:], in_=P['giota'][:])
            brep = pp.tile([128, HID], F32)
            nc.sync.dma_start(out=brep[:], in_=P['brep'][:])
            iota4 = pp.tile([128, 4 * max(Ks)], F32)
            nc.sync.dma_start(out=iota4[:], in_=P['iota4'][:])
            ident = pp.tile([128, 128], F32)
            make_identity(nc, ident[:])
            ones_t = pp.tile([128, 1], BF16)
            nc.vector.memset(ones_t[:], 1.0)

            dinvT = pp.tile([128, NT], F32)
            rcpT = pp.tile([128, NT], F32)
            nc.vector.reciprocal(rcpT[:], degT[:])
            nc.scalar.activation(dinvT[:], rcpT[:], ACT.Sqrt)
            selfb = pp.tile([128, NT * HID], F32)

            for t in range(NT):
                xwp = ps.tile([128, HID], F32, tag="smallps")
                nc.tensor.matmul(xwp[:], xT[:, t * 128:(t + 1) * 128], Wt[:],
                                 start=True, stop=True)
                xw2 = sb.tile([128, HID], F32, tag="xw2")
                nc.vector.tensor_tensor(out=xw2[:], in0=xwp[:],
                                        in1=dinvT[:, t:t + 1].to_broadcast([128, HID]),
                                        op=OP.mult)
                nc.sync.dma_start(out=tloc[t * 128:(t + 1) * 128, :], in_=xw2[:])
                sbt = sb.tile([128, HID], F32, tag="sbt")
                nc.vector.tensor_tensor(out=sbt[:], in0=xw2[:],
                                        in1=dinvT[:, t:t + 1].to_broadcast([128, HID]),
                                        op=OP.mult)
                nc.vector.tensor_add(selfb[:, t * HID:(t + 1) * HID], sbt[:], brep[:])

            nc.gpsimd.collective_compute(
                "AllGather", OP.bypass, replica_groups=groups,
                ins=[tloc[:]], outs=[table[:]])

            # ---------- phase 2: gather + aggregate + pool ----------
            tview = table[:].rearrange("(q s) f -> q (s f)", s=4)
            pool_psum = pps.tile([HID + 1, NGRAPH], F32)
            woff = 0
            ooff = 0
            for t in range(NT):
                K = Ks[t]
                it = gb.tile([128, 8 * max(Ks)], I16, tag="it")
                nc.sync.dma_start(out=it[:, :8 * K], in_=P['slotq'][:, woff:woff + 8 * K])
                sx = gb.tile([128, 4 * max(Ks)], F32, tag="sx")
                nc.sync.dma_start(out=sx[:, :4 * K], in_=P['subx4'][:, ooff:ooff + 4 * K])
                woff += 8 * K
                ooff += 4 * K

                gt = gb.tile([128, max(Ks) * 64], F32, tag="gt")
                nc.gpsimd.dma_gather(
                    gt[:, :K * 64].rearrange("p (c e) -> p c e", e=64),
                    tview, it[:, :8 * K], 128 * K, 128 * K, 64,
                    single_packet=False, queue_num=t % 4)
                msk = sb.tile([128, 4 * max(Ks)], F32, tag="msk")
                nc.vector.tensor_tensor(out=msk[:, :4 * K], in0=sx[:, :4 * K],
                                        in1=iota4[:, :4 * K], op=OP.is_equal)
                mskd = sb.tile([128, max(Ks) * 64], BF16, tag="mskd")
                nc.vector.tensor_tensor(
                    out=mskd[:, :K * 64].rearrange("p (m f) -> p m f", f=HID),
                    in0=gt[:, :K * 64].rearrange("p (m f) -> p m f", f=HID),
                    in1=msk[:, :4 * K].unsqueeze(2).to_broadcast([128, 4 * K, HID]),
                    op=OP.mult)
                nsum = sb.tile([128, HID], F32, tag="nsum")
                nc.vector.tensor_reduce(
                    nsum[:], mskd[:, :K * 64].rearrange("p (m f) -> p f m", f=HID),
                    axis=AX.X, op=OP.add)

                pre = sb.tile([128, HID], F32, tag="pre")
                nc.vector.tensor_tensor(out=pre[:], in0=nsum[:],
                                        in1=dinvT[:, t:t + 1].to_broadcast([128, HID]),
                                        op=OP.mult)
                nc.vector.tensor_add(pre[:], pre[:], selfb[:, t * HID:(t + 1) * HID])
                f17 = sb.tile([128, HID + 1], BF16, tag="f17")
                nc.scalar.activation(f17[:, :HID], pre[:], ACT.Relu)
                nc.vector.tensor_copy(f17[:, HID:HID + 1], ones_t[:])
                G = sb.tile([128, NGRAPH], BF16, tag="G")
                nc.vector.tensor_tensor(out=G[:], in0=gidxT[:, t:t + 1].to_broadcast([128, NGRAPH]),
                                        in1=giota[:], op=OP.is_equal)
                nc.tensor.matmul(pool_psum[:], f17[:], G[:],
                                 start=(t == 0), stop=(t == NT - 1))

            # ---------- phase 3: all-reduce pooled sums; replicated head ----------
            pool_s = sb.tile([HID + 1, NGRAPH], F32)
            nc.vector.tensor_copy(pool_s[:], pool_psum[:])
            nc.sync.dma_start(out=poolloc[:], in_=pool_s[:])
            nc.gpsimd.collective_compute(
                "AllReduce", OP.add, replica_groups=groups,
                ins=[poolloc[:]], outs=[poolred[:]])
            pool_r = pp.tile([HID + 1, NGRAPH], F32)
            nc.sync.dma_start(out=pool_r[:], in_=poolred[:])

            # modality MLPs -> mT tiles [4, 256] in sbuf
            mod_sb = {}
            for m, fdim in MODS:
                mt = pp.tile([fdim if fdim <= 128 else 128,
                              NGRAPH * cdiv(fdim, 128)], F32, tag="mt_" + m)
                if fdim <= 128:
                    nc.sync.dma_start(out=mt[:fdim, :NGRAPH], in_=P[m + 'T'][:])
                else:
                    for k in range(fdim // 128):
                        nc.sync.dma_start(out=mt[:, k * NGRAPH:(k + 1) * NGRAPH],
                                          in_=P[m + 'T'][k * 128:(k + 1) * 128, :])
                wt = pp.tile([fdim if fdim <= 128 else 128,
                              4 * cdiv(fdim, 128)], F32, tag="mw_" + m)
                if fdim <= 128:
                    nc.sync.dma_start(out=wt[:fdim, :4], in_=P[m + 'W'][:])
                else:
                    for k in range(fdim // 128):
                        nc.sync.dma_start(out=wt[:, k * 4:(k + 1) * 4],
                                          in_=P[m + 'W'][k * 128:(k + 1) * 128, :])
                bt = pp.tile([4, 1], F32, tag="mb_" + m)
                nc.sync.dma_start(out=bt[:], in_=P[m + 'b'][:])
                mp = ps.tile([4, NGRAPH], F32, tag="smallps")
                nk = cdiv(fdim, 128)
                for k in range(nk):
                    kk = min(128, fdim - k * 128)
                    nc.tensor.matmul(mp[:], wt[:kk, k * 4:k * 4 + 4],
                                     mt[:kk, k * NGRAPH:(k + 1) * NGRAPH],
                                     start=(k == 0), stop=(k == nk - 1))
                msb = pp.tile([4, NGRAPH], F32, tag="msb_" + m)
                nc.scalar.activation(msb[:], mp[:], ACT.Relu, bias=bt[:])
                mod_sb[m] = msb

            cW1 = pp.tile([32, HID], F32)
            nc.sync.dma_start(out=cW1[:], in_=P['cW1'][:])
            cb1 = pp.tile([HID, 1], F32)
            nc.sync.dma_start(out=cb1[:], in_=P['cb1'][:])
            cW2 = pp.tile([HID, NCLS], F32)
            nc.sync.dma_start(out=cW2[:], in_=P['cW2'][:])
            cb2 = pp.tile([NCLS, 1], F32)
            nc.sync.dma_start(out=cb2[:], in_=P['cb2'][:])

            for gtile in range(2):
                gsl = slice(gtile * 128, (gtile + 1) * 128)
                # transpose pooled sums -> [128 graphs, 17]
                tp = ps.tile([128, HID + 1], F32, tag="smallps")
                nc.tensor.transpose(tp[:], pool_r[:, gsl], ident[:HID + 1, :HID + 1])
                gsum = sb.tile([128, HID + 1], F32, tag="gsum")
                nc.vector.tensor_copy(gsum[:], tp[:])
                cnt = sb.tile([128, 1], F32, tag="cnt")
                nc.vector.tensor_scalar_max(cnt[:], gsum[:, HID:HID + 1], 1.0)
                rec = sb.tile([128, 1], F32, tag="rec")
                nc.vector.reciprocal(rec[:], cnt[:])
                comb = sb.tile([128, 32], F32, tag="comb")
                nc.vector.tensor_scalar_mul(comb[:, :HID], gsum[:, :HID], rec[:])
                coff = HID
                for m, fdim in MODS:
                    mtp = ps.tile([128, 4], F32, tag="smallps")
                    nc.tensor.transpose(mtp[:], mod_sb[m][:, gsl], ident[:4, :4])
                    nc.vector.tensor_copy(comb[:, coff:coff + 4], mtp[:])
                    coff += 4
                # classifier
                ctp = ps.tile([32, 128], F32, tag="smallps")
                nc.tensor.transpose(ctp[:], comb[:], ident[:])
                combT = sb.tile([32, 128], F32, tag="combT")
                nc.vector.tensor_copy(combT[:], ctp[:])
                hp = ps.tile([HID, 128], F32, tag="smallps")
                nc.tensor.matmul(hp[:], cW1[:], combT[:], start=True, stop=True)
                hT = sb.tile([HID, 128], F32, tag="hT")
                nc.scalar.activation(hT[:], hp[:], ACT.Relu, bias=cb1[:])
                lp = ps.tile([NCLS, 128], F32, tag="smallps")
                nc.tensor.matmul(lp[:], cW2[:], hT[:], start=True, stop=True)
                lT = sb.tile([NCLS, 128], F32, tag="lT")
                nc.vector.tensor_scalar_add(lT[:], lp[:], cb2[:])
                ltp = ps.tile([128, NCLS], F32, tag="smallps")
                nc.tensor.transpose(ltp[:], lT[:], ident[:NCLS, :NCLS])
                lg = sb.tile([128, NCLS], F32, tag="lg")
                nc.vector.tensor_copy(lg[:], ltp[:])
                mx = sb.tile([128, 1], F32, tag="mx")
                nc.vector.tensor_reduce(mx[:], lg[:], axis=AX.X, op=OP.max)
                sh = sb.tile([128, NCLS], F32, tag="sh")
                nc.vector.tensor_scalar_sub(sh[:], lg[:], mx[:])
                ex = sb.tile([128, NCLS], F32, tag="ex")
                nc.scalar.activation(ex[:], sh[:], ACT.Exp)
                sm = sb.tile([128, 1], F32, tag="sm")
                nc.vector.tensor_reduce(sm[:], ex[:], axis=AX.X, op=OP.add)
                lns = sb.tile([128, 1], F32, tag="lns")
                nc.scalar.activation(lns[:], sm[:], ACT.Ln)
                fin = sb.tile([128, NCLS], F32, tag="fin")
                nc.vector.tensor_scalar_sub(fin[:], sh[:], lns[:])
                nc.sync.dma_start(out=out[gsl, :], in_=fin[:])
    nc.compile()
    return nc


def kernel(x, edge_index, batch, mri, cog, clin, genetic,
           gcn_W, gcn_b, mri_W, mri_b, cog_W, cog_b, clin_W, clin_b,
           gen_W, gen_b, cls_W1, cls_b1, cls_W2, cls_b2):
    x = np.asarray(x, np.float32)
    src = np.asarray(edge_index[0], np.int64)
    dst = np.asarray(edge_index[1], np.int64)
    batch = np.asarray(batch, np.int64)

    NTOT = NCORE * NLOC
    deg = np.bincount(dst, minlength=NTOT).astype(np.float32) + 1.0
    gid = np.full(NTOT, -1.0, np.float32)
    gid[:N_NODES] = batch.astype(np.float32)

    order = np.argsort(dst, kind='stable')
    src_s = src[order].astype(np.int64)
    counts = np.bincount(dst, minlength=NTOT)
    starts = np.zeros(NTOT + 1, np.int64)
    np.cumsum(counts, out=starts[1:])

    per_core = []
    Ks_all = None
    for c in range(NCORE):
        base = c * NLOC
        Ks = []
        slotw = []
        subw = []
        for t in range(NT):
            nb = base + t * 128
            degs = counts[nb:nb + 128]
            K = max(1, int(degs.max()))
            Ks.append(K)
            qa = np.zeros((K * 128,), np.int16)
            sa = np.full((128, K), -1.0, np.float32)
            for p in range(128):
                n = nb + p
                s0, s1 = starts[n], starts[n + 1]
                e = src_s[s0:s1]
                kk = s1 - s0
                qa[np.arange(kk) * 128 + p] = (e // 4).astype(np.int16)
                sa[p, :kk] = (e % 4).astype(np.float32)
            iw = np.tile(qa.reshape(8 * K, 16).T, (8, 1))  # [128, 8K]
            slotw.append(iw.astype(np.int16))
            subw.append(np.repeat(sa, 4, axis=1))  # [128, 4K]
        if Ks_all is None:
            Ks_all = Ks
        else:
            Ks_all = [max(a, b) for a, b in zip(Ks_all, Ks)]
        per_core.append((slotw, subw, Ks))

    # unify K per tile across cores so one program serves all cores
    Ks = Ks_all
    totw = sum(8 * k for k in Ks)
    tot4 = sum(4 * k for k in Ks)
    in_maps = []
    giota = np.tile(np.arange(NGRAPH, dtype=np.float32), (128, 1))
    iota4 = np.tile(np.tile(np.arange(4, dtype=np.float32), max(Ks)), (128, 1))
    for c in range(NCORE):
        slotw, subw, Kc = per_core[c]
        sq = np.zeros((128, totw), np.int16)
        s4 = np.full((128, tot4), -1.0, np.float32)
        wo = 0
        oo = 0
        for t in range(NT):
            K, Kcur = Ks[t], Kc[t]
            sq[:, wo:wo + 8 * Kcur] = slotw[t]
            s4[:, oo:oo + 4 * Kcur] = subw[t]
            wo += 8 * K
            oo += 4 * K
        base = c * NLOC
        xs = np.zeros((NLOC, NODE_DIM), np.float32)
        ncopy = min(NLOC, N_NODES - base) if base < N_NODES else 0
        if ncopy > 0:
            xs[:ncopy] = x[base:base + ncopy]
        m = {
            'x16': xs.astype(ml_dtypes.bfloat16),
            'W16': np.asarray(gcn_W, np.float32).astype(ml_dtypes.bfloat16),
            'degT': deg[base:base + NLOC].reshape(NT, 128).T.copy(),
            'gidxT': gid[base:base + NLOC].reshape(NT, 128).T.copy(),
            'giota': giota,
            'brep': np.tile(np.asarray(gcn_b, np.float32), (128, 1)),
            'slotq': sq, 'subx4': s4, 'iota4': iota4,
            'cW1': np.asarray(cls_W1, np.float32),
            'cb1': np.asarray(cls_b1, np.float32).reshape(-1, 1),
            'cW2': np.asarray(cls_W2, np.float32),
            'cb2': np.asarray(cls_b2, np.float32).reshape(-1, 1),
        }
        for (mn, fdim), mv, wv, bv in zip(
                MODS, (mri, cog, clin, genetic),
                (mri_W, cog_W, clin_W, gen_W), (mri_b, cog_b, clin_b, gen_b)):
            m[mn + 'T'] = np.asarray(mv, np.float32).T.copy()
            m[mn + 'W'] = np.asarray(wv, np.float32)
            m[mn + 'b'] = np.asarray(bv, np.float32).reshape(-1, 1)
        in_maps.append(m)

    nc = _build(Ks, totw, tot4)
    res = run_bass_kernel_spmd(nc, in_maps, core_ids=list(range(NCORE)))
    return res.results[0]["out"].astype(np.float32)


# revision 7
# speedup vs baseline: 1.0368x; 1.0368x over previous
import sys
if '/opt/trn_rl_repo' not in sys.path:
    sys.path.insert(0, '/opt/trn_rl_repo')
import numpy as np
import ml_dtypes

import concourse.bass as bass
import concourse.bacc as bacc
import concourse.mybir as mybir
import concourse.tile as tile
from concourse import library_config
from concourse.masks import make_identity
from concourse.bass_utils import run_bass_kernel_spmd
from concourse._compat import cdiv

NCORE = 8
N_NODES = 100000
N_EDGES = 3200000
NODE_DIM = 128
HID = 16
NGRAPH = 256
NCLS = 3
NLOC = 12544            # nodes per core (98 tiles of 128); 8*12544 = 100352
NT = NLOC // 128        # 98 tiles per core
NQUAD = (NCORE * NLOC) // 4
MODS = [("mri", 256), ("cog", 64), ("clin", 32), ("gen", 512)]

F32 = mybir.dt.float32
BF16 = mybir.dt.bfloat16
I16 = mybir.dt.int16
AX = mybir.AxisListType
OP = mybir.AluOpType
ACT = mybir.ActivationFunctionType


def _build(Ks, totw, tot4):
    """Build the per-core SPMD program. Ks: per-tile slot counts (same for
    every core by construction), totw/tot4: flat widths of the index arrays."""
    nc = bacc.Bacc(num_swdge_queues=4)
    P = {}
    P['x16'] = nc.declare_dram_parameter("x16", [NLOC, NODE_DIM], BF16, isOutput=False)
    P['W16'] = nc.declare_dram_parameter("W16", [NODE_DIM, HID], BF16, isOutput=False)
    P['degT'] = nc.declare_dram_parameter("degT", [128, NT], F32, isOutput=False)
    P['gidxT'] = nc.declare_dram_parameter("gidxT", [128, NT], F32, isOutput=False)
    P['giota'] = nc.declare_dram_parameter("giota", [128, NGRAPH], F32, isOutput=False)
    P['brep'] = nc.declare_dram_parameter("brep", [128, HID], F32, isOutput=False)
    P['slotq'] = nc.declare_dram_parameter("slotq", [128, totw], I16, isOutput=False)
    P['subx4'] = nc.declare_dram_parameter("subx4", [128, tot4], F32, isOutput=False)
    P['iota4'] = nc.declare_dram_parameter("iota4", [128, 4 * max(Ks)], F32, isOutput=False)
    for m, fdim in MODS:
        P[m + 'T'] = nc.declare_dram_parameter(m + 'T', [fdim, NGRAPH], F32, isOutput=False)
        P[m + 'W'] = nc.declare_dram_parameter(m + 'W', [fdim, 4], F32, isOutput=False)
        P[m + 'b'] = nc.declare_dram_parameter(m + 'b', [4, 1], F32, isOutput=False)
    P['cW1'] = nc.declare_dram_parameter("cW1", [32, HID], F32, isOutput=False)
    P['cb1'] = nc.declare_dram_parameter("cb1", [HID, 1], F32, isOutput=False)
    P['cW2'] = nc.declare_dram_parameter("cW2", [HID, NCLS], F32, isOutput=False)
    P['cb2'] = nc.declare_dram_parameter("cb2", [NCLS, 1], F32, isOutput=False)
    out = nc.declare_dram_parameter("out", [NGRAPH, NCLS], F32, isOutput=True)

    tloc = nc.dram_tensor("tloc", [NLOC, HID], F32)
    table = nc.dram_tensor("table", [NCORE * NLOC, HID], F32, addr_space="Shared")
    poolloc = nc.dram_tensor("poolloc", [HID + 1, NGRAPH], F32)
    poolred = nc.dram_tensor("poolred", [HID + 1, NGRAPH], F32, addr_space="Shared")
    groups = [list(range(NCORE))]

    with tile.TileContext(nc) as tc:
        with tc.tile_pool(name="pers", bufs=1) as pp, \
             tc.tile_pool(name="sb", bufs=2) as sb, \
             tc.tile_pool(name="gat", bufs=6) as gb, \
             tc.tile_pool(name="ps", bufs=2, space="PSUM") as ps, \
             tc.tile_pool(name="pool_ps", bufs=1, space="PSUM") as pps:
            nc.gpsimd.load_library(library_config.mlp)

            # ---------- phase 1: local xw2 shard + AllGather table ----------
            xT = pp.tile([128, NLOC], BF16)
            nc.sync.dma_start_transpose(xT[:], P['x16'][:])
            Wt = pp.tile([NODE_DIM, HID], BF16)
            nc.sync.dma_start(out=Wt[:], in_=P['W16'][:])
            degT = pp.tile([128, NT], F32)
            nc.sync.dma_start(out=degT[:], in_=P['degT'][:])
            gidxT = pp.tile([128, NT], F32)
            nc.sync.dma_start(out=gidxT[:], in_=P['gidxT'][:])
            giota = pp.tile([128, NGRAPH], F32)
            nc.sync.dma_start(out=giota[:], in_=P['giota'][:])
            brep = pp.tile([128, HID], F32)
            nc.sync.dma_start(out=brep[:], in_=P['brep'][:])
            iota4 = pp.tile([128, 4 * max(Ks)], F32)
            nc.sync.dma_start(out=iota4[:], in_=P['iota4'][:])
            ident = pp.tile([128, 128], F32)
            make_identity(nc, ident[:])
            ones_t = pp.tile([128, 1], BF16)
            nc.vector.memset(ones_t[:], 1.0)

            dinvT = pp.tile([128, NT], F32)
            rcpT = pp.tile([128, NT], F32)
            nc.vector.reciprocal(rcpT[:], degT[:])
            nc.scalar.activation(dinvT[:], rcpT[:], ACT.Sqrt)
            selfb = pp.tile([128, NT * HID], F32)

            for t in range(NT):
                xwp = ps.tile([128, HID], F32, tag="smallps")
                nc.tensor.matmul(xwp[:], xT[:, t * 128:(t + 1) * 128], Wt[:],
                                 start=True, stop=True)
                xw2 = sb.tile([128, HID], F32, tag="xw2")
                nc.vector.tensor_tensor(out=xw2[:], in0=xwp[:],
                                        in1=dinvT[:, t:t + 1].to_broadcast([128, HID]),
                                        op=OP.mult)
                nc.sync.dma_start(out=tloc[t * 128:(t + 1) * 128, :], in_=xw2[:])
                sbt = sb.tile([128, HID], F32, tag="sbt")
                nc.vector.tensor_tensor(out=sbt[:], in0=xw2[:],
                                        in1=dinvT[:, t:t + 1].to_broadcast([128, HID]),
                                        op=OP.mult)
                nc.vector.tensor_add(selfb[:, t * HID:(t + 1) * HID], sbt[:], brep[:])

            nc.gpsimd.collective_compute(
                "AllGather", OP.bypass, replica_groups=groups,
                ins=[tloc[:]], outs=[table[:]])

            # ---------- phase 2: gather + aggregate + pool ----------
            tview = table[:].rearrange("(q s) f -> q (s f)", s=4)
            pool_psum = pps.tile([HID + 1, NGRAPH], F32)
            woff = 0
            ooff = 0
            for t in range(NT):
                K = Ks[t]
                it = gb.tile([128, 8 * max(Ks)], I16, tag="it")
                nc.sync.dma_start(out=it[:, :8 * K], in_=P['slotq'][:, woff:woff + 8 * K])
                sx = gb.tile([128, 4 * max(Ks)], F32, tag="sx")
                nc.sync.dma_start(out=sx[:, :4 * K], in_=P['subx4'][:, ooff:ooff + 4 * K])
                woff += 8 * K
                ooff += 4 * K

                gt = gb.tile([128, max(Ks) * 64], F32, tag="gt")
                nc.gpsimd.dma_gather(
                    gt[:, :K * 64].rearrange("p (c e) -> p c e", e=64),
                    tview, it[:, :8 * K], 128 * K, 128 * K, 64,
                    single_packet=False, queue_num=t % 4)
                msk = sb.tile([128, 4 * max(Ks)], F32, tag="msk")
                nc.vector.tensor_tensor(out=msk[:, :4 * K], in0=sx[:, :4 * K],
                                        in1=iota4[:, :4 * K], op=OP.is_equal)
                mskd = sb.tile([128, max(Ks) * 64], BF16, tag="mskd")
                nc.vector.tensor_tensor(
                    out=mskd[:, :K * 64].rearrange("p (m f) -> p m f", f=HID),
                    in0=gt[:, :K * 64].rearrange("p (m f) -> p m f", f=HID),
                    in1=msk[:, :4 * K].unsqueeze(2).to_broadcast([128, 4 * K, HID]),
                    op=OP.mult)
                nsum = sb.tile([128, HID], F32, tag="nsum")
                nc.vector.tensor_reduce(
                    nsum[:], mskd[:, :K * 64].rearrange("p (m f) -> p f m", f=HID),
                    axis=AX.X, op=OP.add)

                pre = sb.tile([128, HID], F32, tag="pre")
                nc.vector.tensor_tensor(out=pre[:], in0=nsum[:],
                                        in1=dinvT[:, t:t + 1].to_broadcast([128, HID]),
                                        op=OP.mult)
                nc.vector.tensor_add(pre[:], pre[:], selfb[:, t * HID:(t + 1) * HID])
                f17 = sb.tile([128, HID + 1], BF16, tag="f17")
                nc.scalar.activation(f17[:, :HID], pre[:], ACT.Relu)
                nc.vector.tensor_copy(f17[:, HID:HID + 1], ones_t[:])
                G = sb.tile([128, NGRAPH], BF16, tag="G")
                nc.vector.tensor_tensor(out=G[:], in0=gidxT[:, t:t + 1].to_broadcast([128, NGRAPH]),
                                        in1=giota[:], op=OP.is_equal)
                nc.tensor.matmul(pool_psum[:], f17[:], G[:],
                                 start=(t == 0), stop=(t == NT - 1))

            # ---------- phase 3: all-reduce pooled sums; replicated head ----------
            pool_s = sb.tile([HID + 1, NGRAPH], F32)
            nc.vector.tensor_copy(pool_s[:], pool_psum[:])
            nc.sync.dma_start(out=poolloc[:], in_=pool_s[:])
            nc.gpsimd.collective_compute(
                "AllReduce", OP.add, replica_groups=groups,
                ins=[poolloc[:]], outs=[poolred[:]])
            pool_r = pp.tile([HID + 1, NGRAPH], F32)
            nc.sync.dma_start(out=pool_r[:], in_=poolred[:])

            # modality MLPs -> mT tiles [4, 256] in sbuf
            mod_sb = {}
            for m, fdim in MODS:
                mt = pp.tile([fdim if fdim <= 128 else 128,
                              NGRAPH * cdiv(fdim, 128)], F32, tag="mt_" + m)
                if fdim <= 128:
                    nc.sync.dma_start(out=mt[:fdim, :NGRAPH], in_=P[m + 'T'][:])
                else:
                    for k in range(fdim // 128):
                        nc.sync.dma_start(out=mt[:, k * NGRAPH:(k + 1) * NGRAPH],
                                          in_=P[m + 'T'][k * 128:(k + 1) * 128, :])
                wt = pp.tile([fdim if fdim <= 128 else 128,
                              4 * cdiv(fdim, 128)], F32, tag="mw_" + m)
                if fdim <= 128:
                    nc.sync.dma_start(out=wt[:fdim, :4], in_=P[m + 'W'][:])
                else:
                    for k in range(fdim // 128):
                        nc.sync.dma_start(out=wt[:, k * 4:(k + 1) * 4],
                                          in_=P[m + 'W'][k * 128:(k + 1) * 128, :])
                bt = pp.tile([4, 1], F32, tag="mb_" + m)
                nc.sync.dma_start(out=bt[:], in_=P[m + 'b'][:])
                mp = ps.tile([4, NGRAPH], F32, tag="smallps")
                nk = cdiv(fdim, 128)
                for k in range(nk):
                    kk = min(128, fdim - k * 128)
                    nc.tensor.matmul(mp[:], wt[:kk, k * 4:k * 4 + 4],
                                     mt[:kk, k * NGRAPH:(k + 1) * NGRAPH],
                                     start=(k == 0), stop=(k == nk - 1))
                msb = pp.tile([4, NGRAPH], F32, tag="msb_" + m)
                nc.scalar.activation(msb[:], mp[:], ACT.Relu, bias=bt[:])
                mod_sb[m] = msb

            cW1 = pp.tile([32, HID], F32)
            nc.sync.dma_start(out=cW1[:], in_=P['cW1'][:])
            cb1 = pp.tile([HID, 1], F32)
            nc.sync.dma_start(out=cb1[:], in_=P['cb1'][:])
            cW2 = pp.tile([HID, NCLS], F32)
            nc.sync.dma_start(out=cW2[:], in_=P['cW2'][:])
            cb2 = pp.tile([NCLS, 1], F32)
            nc.sync.dma_start(out=cb2[:], in_=P['cb2'][:])

            for gtile in range(2):
                gsl = slice(gtile * 128, (gtile + 1) * 128)
                # transpose pooled sums -> [128 graphs, 17]
                tp = ps.tile([128, HID + 1], F32, tag="smallps")
                nc.tensor.transpose(tp[:], pool_r[:, gsl], ident[:HID + 1, :HID + 1])
                gsum = sb.tile([128, HID + 1], F32, tag="gsum")
                nc.vector.tensor_copy(gsum[:], tp[:])
                cnt = sb.tile([128, 1], F32, tag="cnt")
                nc.vector.tensor_scalar_max(cnt[:], gsum[:, HID:HID + 1], 1.0)
                rec = sb.tile([128, 1], F32, tag="rec")
                nc.vector.reciprocal(rec[:], cnt[:])
                comb = sb.tile([128, 32], F32, tag="comb")
                nc.vector.tensor_scalar_mul(comb[:, :HID], gsum[:, :HID], rec[:])
                coff = HID
                for m, fdim in MODS:
                    mtp = ps.tile([128, 4], F32, tag="smallps")
                    nc.tensor.transpose(mtp[:], mod_sb[m][:, gsl], ident[:4, :4])
                    nc.vector.tensor_copy(comb[:, coff:coff + 4], mtp[:])
                    coff += 4
                # classifier
                ctp = ps.tile([32, 128], F32, tag="smallps")
                nc.tensor.transpose(ctp[:], comb[:], ident[:])
                combT = sb.tile([32, 128], F32, tag="combT")
                nc.vector.tensor_copy(combT[:], ctp[:])
                hp = ps.tile([HID, 128], F32, tag="smallps")
                nc.tensor.matmul(hp[:], cW1[:], combT[:], start=True, stop=True)
                hT = sb.tile([HID, 128], F32, tag="hT")
                nc.scalar.activation(hT[:], hp[:], ACT.Relu, bias=cb1[:])
                lp = ps.tile([NCLS, 128], F32, tag="smallps")
                nc.tensor.matmul(lp[:], cW2[:], hT[:], start=True, stop=True)
                lT = sb.tile([NCLS, 128], F32, tag="lT")
                nc.vector.tensor_scalar_add(lT[:], lp[:], cb2[:])
                ltp = ps.tile([128, NCLS], F32, tag="smallps")
                nc.tensor.transpose(ltp[:], lT[:], ident[:NCLS, :NCLS])
                lg = sb.tile([128, NCLS], F32, tag="lg")
                nc.vector.tensor_copy(lg[:], ltp[:])
                mx = sb.tile([128, 1], F32, tag="mx")
                nc.vector.tensor_reduce(mx[:], lg[:], axis=AX.X, op=OP.max)
                sh = sb.tile([128, NCLS], F32, tag="sh")
                nc.vector.tensor_scalar_sub(sh[:], lg[:], mx[:])
                ex = sb.tile([128, NCLS], F32, tag="ex")
                nc.scalar.activation(ex[:], sh[:], ACT.Exp)
                sm = sb.tile([128, 1], F32, tag="sm")
                nc.vector.tensor_reduce(sm[:], ex[:], axis=AX.X, op=OP.add)
                lns = sb.tile([128, 1], F32, tag="lns")
                nc.scalar.activation(lns[:], sm[:], ACT.Ln)
                fin = sb.tile([128, NCLS], F32, tag="fin")
                nc.vector.tensor_scalar_sub(fin[:], sh[:], lns[:])
                nc.sync.dma_start(out=out[gsl, :], in_=fin[:])
    nc.compile()
    return nc


def kernel(x, edge_index, batch, mri, cog, clin, genetic,
           gcn_W, gcn_b, mri_W, mri_b, cog_W, cog_b, clin_W, clin_b,
           gen_W, gen_b, cls_W1, cls_b1, cls_W2, cls_b2):
    x = np.asarray(x, np.float32)
    src = np.asarray(edge_index[0], np.int64)
    dst = np.asarray(edge_index[1], np.int64)
    batch = np.asarray(batch, np.int64)

    NTOT = NCORE * NLOC
    deg = np.bincount(dst, minlength=NTOT).astype(np.float32) + 1.0
    gid = np.full(NTOT, -1.0, np.float32)
    gid[:N_NODES] = batch.astype(np.float32)

    order = np.argsort(dst, kind='stable')
    src_s = src[order].astype(np.int64)
    counts = np.bincount(dst, minlength=NTOT)
    starts = np.zeros(NTOT + 1, np.int64)
    np.cumsum(counts, out=starts[1:])

    per_core = []
    Ks_all = None
    for c in range(NCORE):
        base = c * NLOC
        Ks = []
        slotw = []
        subw = []
        for t in range(NT):
            nb = base + t * 128
            degs = counts[nb:nb + 128]
            K = max(1, int(degs.max()))
            Ks.append(K)
            qa = np.zeros((K * 128,), np.int16)
            sa = np.full((128, K), -1.0, np.float32)
            for p in range(128):
                n = nb + p
                s0, s1 = starts[n], starts[n + 1]
                e = src_s[s0:s1]
                kk = s1 - s0
                qa[np.arange(kk) * 128 + p] = (e // 4).astype(np.int16)
                sa[p, :kk] = (e % 4).astype(np.float32)
            iw = np.tile(qa.reshape(8 * K, 16).T, (8, 1))  # [128, 8K]
            slotw.append(iw.astype(np.int16))
            subw.append(np.repeat(sa, 4, axis=1))  # [128, 4K]
        if Ks_all is None:
            Ks_all = Ks
        else:
            Ks_all = [max(a, b) for a, b in zip(Ks_all, Ks)]
        per_core.append((slotw, subw, Ks))

    # unify K per tile across cores so one program serves all cores
    Ks = Ks_all
    totw = sum(8 * k for k in Ks)
    tot4 = sum(4 * k for k in Ks)
    in_maps = []
    giota = np.tile(np.arange(NGRAPH, dtype=np.float32), (128, 1))
    iota4 = np.tile(np.tile(np.arange(4, dtype=np.float32), max(Ks)), (128, 1))
    for c in range(NCORE):
        slotw, subw, Kc = per_core[c]
        sq = np.zeros((128, totw), np.int16)
        s4 = np.full((128, tot4), -1.0, np.float32)
        wo = 0
        oo = 0
        for t in range(NT):
            K, Kcur = Ks[t], Kc[t]
            sq[:, wo:wo + 8 * Kcur] = slotw[t]
            s4[:, oo:oo + 4 * Kcur] = subw[t]
            wo += 8 * K
            oo += 4 * K
        base = c * NLOC
        xs = np.zeros((NLOC, NODE_DIM), np.float32)
        ncopy = min(NLOC, N_NODES - base) if base < N_NODES else 0
        if ncopy > 0:
            xs[:ncopy] = x[base:base + ncopy]
        m = {
            'x16': xs.astype(ml_dtypes.bfloat16),
            'W16': np.asarray(gcn_W, np.float32).astype(ml_dtypes.bfloat16),
            'degT': deg[base:base + NLOC].reshape(NT, 128).T.copy(),
            'gidxT': gid[base:base + NLOC].reshape(NT, 128).T.copy(),
            'giota': giota,
            'brep': np.tile(np.asarray(gcn_b, np.float32), (128, 1)),
            'slotq': sq, 'subx4': s4, 'iota4': iota4,
            'cW1': np.asarray(cls_W1, np.float32),
            'cb1': np.asarray(cls_b1, np.float32).reshape(-1, 1),
            'cW2': np.asarray(cls_W2, np.float32),
            'cb2': np.asarray(cls_b2, np.float32).reshape(-1, 1),
        }
        for (mn, fdim), mv, wv, bv in zip(
                MODS, (mri, cog, clin, genetic),
                (mri_W, cog_W, clin_W, gen_W), (mri_b, cog_b, clin_b, gen_b)):
            m[mn + 'T'] = np.asarray(mv, np.float32).T.copy()
            m[mn + 'W'] = np.asarray(wv, np.float32)
            m[mn + 'b'] = np.asarray(bv, np.float32).reshape(-1, 1)
        in_maps.append(m)

    nc = _build(Ks, totw, tot4)
    res = run_bass_kernel_spmd(nc, in_maps, core_ids=list(range(NCORE)))
    return res.results[0]["out"].astype(np.float32)
